# revision 5
# baseline (speedup 1.0000x reference)
"""Bidirectional cross-attention kernel for 8 Trainium2 NeuronCores.

Sharding: core c = 2*b + g handles batch b with head-group g (8 of 16 heads).
v2 restructure vs baseline:
 - sim matmuls are row-tiled: the two heads of a pair run CONCURRENTLY in the
   128x128 PE array (K=64 each, tile rows 0:64 / 64:128) instead of being
   zero-padded to K=128.  Halves sim PE time.
 - U/W accumulations are col-tiled: both heads' outputs pack into one
   [128, N] PSUM tile (M=64 each, array cols 0:64 / 64:128).  Halves U/W PE
   time and drops the ones-column; softmax normalizers instead come free from
   the exp activation's accum_out (free-dim running sum).
 - Projections run head-pair-major (m-outer) and are WOVEN between attention
   chunks so the PE back-fills the exp-shadow; attention starts as soon as
   the first pair's Q/K projections land (~15us instead of ~88us).
 - Normalization: reciprocal of the accum sums, DMA round-trip through DRAM
   to transpose/broadcast (normd row holds r[i] i-linear), then one
   tensor_mul per (pair, side).
"""

import os
import sys

import numpy as np

for _p in ("/opt/trn_rl_repo", "/root/.axon_site/_ro/trn_rl_repo"):
    if os.path.isdir(_p) and _p not in sys.path:
        sys.path.append(_p)

import ml_dtypes  # noqa: E402
import concourse.bass as bass  # noqa: E402
import concourse.mybir as mybir  # noqa: E402
import concourse.tile as tile  # noqa: E402
from concourse import bacc  # noqa: E402
from concourse.bass_utils import run_bass_kernel_spmd  # noqa: E402

B, N, DIM = 4, 1024, 1024
H, DH = 16, 64
HL = 8            # heads per core
IL = HL * DH      # local inner width (512)
COLS = 512        # output columns per core
P = 128
PAIRS = HL // 2   # head pairs per core
KCH = DIM // P    # contraction chunks (8)
ICH = N // P      # sequence chunks (8)
SCALE = DH ** -0.5
GROUPS = [[0, 1], [2, 3], [4, 5], [6, 7]]

F32 = mybir.dt.float32
BF16 = mybir.dt.bfloat16
EXP = mybir.ActivationFunctionType.Exp

_CACHED_NC = None


def _build_nc():
    nc = bacc.Bacc("TRN2", target_bir_lowering=False, debug=False, num_devices=8)

    xT = nc.dram_tensor("xT", [DIM, N], BF16, kind="ExternalInput")
    ctxT = nc.dram_tensor("ctxT", [DIM, N], BF16, kind="ExternalInput")
    # wqk/cwqk fed m-major: [4*DIM, 128], block m at rows m*DIM:(m+1)*DIM
    wqk = nc.dram_tensor("wqk", [4 * DIM, P], BF16, kind="ExternalInput")
    cwqk = nc.dram_tensor("cwqk", [4 * DIM, P], BF16, kind="ExternalInput")
    wv = nc.dram_tensor("wv", [DIM, IL], BF16, kind="ExternalInput")
    cwv = nc.dram_tensor("cwv", [DIM, IL], BF16, kind="ExternalInput")
    wout = nc.dram_tensor("wout", [DIM, COLS], BF16, kind="ExternalInput")
    cwout = nc.dram_tensor("cwout", [DIM, COLS], BF16, kind="ExternalInput")
    bout = nc.dram_tensor("bout", [1, COLS], F32, kind="ExternalInput")
    cbout = nc.dram_tensor("cbout", [1, COLS], F32, kind="ExternalInput")

    out_cols = nc.dram_tensor("out_cols", [N, COLS], F32, kind="ExternalOutput")
    ctx_cols = nc.dram_tensor("ctx_cols", [N, COLS], F32, kind="ExternalOutput")

    with tile.TileContext(nc) as tc:
        with tc.tile_pool(name="dram", bufs=1, space="DRAM") as dpool:
            uwl = [dpool.tile([256, N], BF16, tag=f"uwl{p}", name=f"uwl{p}")
                   for p in range(4)]
            uwa = [dpool.tile([512, N], BF16, tag=f"uwa{p}", name=f"uwa{p}")
                   for p in range(4)]
            normd = dpool.tile([16, N], F32, tag="normd")
            _build_body(nc, tc, dict(
                xT=xT, ctxT=ctxT, wqk=wqk, wv=wv, cwqk=cwqk, cwv=cwv,
                wout=wout, cwout=cwout, bout=bout, cbout=cbout,
                out_cols=out_cols, ctx_cols=ctx_cols,
                uwl=uwl, uwa=uwa, normd=normd,
            ))
    nc.compile()
    if os.environ.get("KERNEL_LDW_DEDUP", "1") == "1":
        _dedupe_ldweights(nc)
    return nc


def _dedupe_ldweights(nc):
    """Drop PE Ldweights that reload weights already resident in their
    row/col tile group.  Tracks the last-loaded signature PER tile group and
    invalidates on overlap, so the A1,B1,A2,B2 interleave of concurrent
    tiled matmuls still dedupes A2/B2's reloads."""
    def sig(i):
        a = i.ins[0]
        return (a.memref, a.offset, str(a.ap), str(a.dtype),
                str(i.perf_mode), str(i.is_transpose))

    def region(i):
        tp = i.tile_position or (0, 0)
        ts = i.tile_size or (128, 128)
        return (tp[0], tp[0] + ts[0], tp[1], tp[1] + ts[1])

    def overlap(r1, r2):
        return r1[0] < r2[1] and r2[0] < r1[1] and r1[2] < r2[3] and r2[2] < r1[3]

    removed = 0
    for fn in nc.m.functions:
        for bb in fn.blocks:
            last = {}
            keep = []
            for i in bb.instructions:
                if isinstance(i, mybir.InstLdweights):
                    s = sig(i)
                    r = region(i)
                    key = (str(i.tile_position), str(i.tile_size))
                    si = i.sync_info
                    clean = si is None or (not si.on_wait and not si.on_update)
                    if clean and last.get(key, (None, None))[0] == s:
                        removed += 1
                        continue
                    for k2 in list(last):
                        if k2 != key and overlap(last[k2][1], r):
                            del last[k2]
                    last[key] = (s, r)
                elif isinstance(i, mybir.InstMatmult):
                    pass
                elif getattr(i, "engine", None) == mybir.EngineType.PE:
                    last = {}
                keep.append(i)
            if removed:
                bb.instructions = keep
    return removed


def _build_body(nc, tc, T):
    from contextlib import ExitStack
    stack = ExitStack()
    pqk = stack.enter_context(tc.tile_pool(name="pqk", bufs=1))
    pv = stack.enter_context(tc.tile_pool(name="pv", bufs=1))
    pf = stack.enter_context(tc.tile_pool(name="pf", bufs=1))
    pu = stack.enter_context(tc.tile_pool(name="pu", bufs=1))
    pn = stack.enter_context(tc.tile_pool(name="pn", bufs=1))
    pe = stack.enter_context(tc.tile_pool(name="pe", bufs=5))
    pub = stack.enter_context(tc.tile_pool(name="pub", bufs=1))
    # closed before the final projections to free SBUF + PSUM
    stmid = ExitStack()
    pin = stmid.enter_context(tc.tile_pool(name="pin", bufs=1))
    pw = stmid.enter_context(tc.tile_pool(name="pw", bufs=1))
    psS = stmid.enter_context(tc.tile_pool(name="psS", bufs=2, space="PSUM"))
    psUW = stmid.enter_context(tc.tile_pool(name="psUW", bufs=1, space="PSUM"))
    psP = stmid.enter_context(tc.tile_pool(name="psP", bufs=2, space="PSUM"))

    # ---- ACT table warmup: dummy exp so the table set loads during DMA ----
    warm = pn.tile([1, 8], F32, tag="warm")
    nc.vector.memset(warm[:], 0.0)
    warm2 = pn.tile([1, 8], F32, tag="warm2")
    nc.scalar.activation(warm2[:], warm[:], EXP)

    # ---- input DMAs (sync queue: x-side; scalar queue: context-side) ----
    xt, ct = [], []
    wqk_sb = [[None] * KCH for _ in range(4)]
    cwqk_sb = [[None] * KCH for _ in range(4)]
    for k in range(KCH):
        t = pin.tile([P, N], BF16, tag=f"xT{k}")
        nc.sync.dma_start(t[:], T["xT"][k * P:(k + 1) * P, :])
        xt.append(t)
        t = pin.tile([P, N], BF16, tag=f"cT{k}")
        nc.scalar.dma_start(t[:], T["ctxT"][k * P:(k + 1) * P, :])
        ct.append(t)

    def load_wqk_m(m):
        for k in range(KCH):
            t = pw.tile([P, P], BF16, tag=f"wq{m}_{k}")
            nc.sync.dma_start(t[:], T["wqk"][m * DIM + k * P:m * DIM + (k + 1) * P, :])
            wqk_sb[m][k] = t
            t = pw.tile([P, P], BF16, tag=f"cwq{m}_{k}")
            nc.scalar.dma_start(t[:], T["cwqk"][m * DIM + k * P:m * DIM + (k + 1) * P, :])
            cwqk_sb[m][k] = t

    load_wqk_m(0)
    wv_t, cwv_t = [], []
    for k in range(KCH):
        t = pw.tile([P, IL], BF16, tag=f"wv{k}")
        nc.sync.dma_start(t[:], T["wv"][k * P:(k + 1) * P, :])
        wv_t.append(t)
        t = pw.tile([P, IL], BF16, tag=f"cwv{k}")
        nc.scalar.dma_start(t[:], T["cwv"][k * P:(k + 1) * P, :])
        cwv_t.append(t)
    for m in (1, 2, 3):
        load_wqk_m(m)

    # output-side weights/biases
    bout_bc = pf.tile([P, COLS], F32, tag="bb")
    nc.sync.dma_start(bout_bc[:], T["bout"][:].to_broadcast((P, COLS)))
    cbout_bc = pf.tile([P, COLS], F32, tag="cbb")
    nc.scalar.dma_start(cbout_bc[:], T["cbout"][:].to_broadcast((P, COLS)))
    wout_sb, cwout_sb = [], []
    for k in range(KCH):
        t = pf.tile([P, COLS], BF16, tag=f"wo{k}")
        nc.sync.dma_start(t[:], T["wout"][k * P:(k + 1) * P, :])
        wout_sb.append(t)
        t = pf.tile([P, COLS], BF16, tag=f"cwo{k}")
        nc.scalar.dma_start(t[:], T["cwout"][k * P:(k + 1) * P, :])
        cwout_sb.append(t)

    QTm = [None] * 4
    KTm = [None] * 4
    V = [None] * ICH
    CV = [None] * ICH

    def proj_qt(which, m):
        """[128, N] tile: rows 0:64 head 2m, 64:128 head 2m+1 (transposed)."""
        src = xt if which == "q" else ct
        wsb = wqk_sb[m] if which == "q" else cwqk_sb[m]
        ps_a = psP.tile([P, COLS], F32, tag="pp", name=f"p{which}{m}a")
        ps_b = psP.tile([P, COLS], F32, tag="pp", name=f"p{which}{m}b")
        for k in range(KCH):
            nc.tensor.matmul(ps_a[:], wsb[k][:], src[k][:, 0:512],
                             start=(k == 0), stop=(k == KCH - 1))
            nc.tensor.matmul(ps_b[:], wsb[k][:], src[k][:, 512:1024],
                             start=(k == 0), stop=(k == KCH - 1))
        t = pqk.tile([P, N], BF16, tag=f"{which}t{m}")
        nc.vector.tensor_copy(t[:, 0:512], ps_a[:])
        nc.vector.tensor_copy(t[:, 512:1024], ps_b[:])
        if which == "q":
            QTm[m] = t
        else:
            KTm[m] = t

    def proj_v(which, ic):
        """[128, 512] tile: seq chunk ic rows, head h at cols h*64:(h+1)*64."""
        src = xt if which == "v" else ct
        wsb = wv_t if which == "v" else cwv_t
        ps = psP.tile([P, COLS], F32, tag="pp", name=f"p{which}{ic}")
        for k in range(KCH):
            nc.tensor.matmul(ps[:], src[k][:, ic * P:(ic + 1) * P], wsb[k][:],
                             start=(k == 0), stop=(k == KCH - 1))
        o = pv.tile([P, IL], BF16, tag=f"{which}{ic}")
        nc.vector.tensor_copy(o[:], ps[:])
        if which == "v":
            V[ic] = o
        else:
            CV[ic] = o

    u_sb = [None] * KCH
    w_sb = [None] * KCH

    def load_uw(k, src_rows, u_off, w_off):
        t = pu.tile([P, N], BF16, tag=f"ua{k}")
        nc.sync.dma_start(t[:], src_rows[u_off:u_off + P, :])
        u_sb[k] = t
        t = pu.tile([P, N], BF16, tag=f"wa{k}")
        nc.scalar.dma_start(t[:], src_rows[w_off:w_off + P, :])
        w_sb[k] = t

    def pair_attention(p, weave):
        """weave: list of callables, one popped per chunk slot."""
        hA, hB = 2 * p, 2 * p + 1
        cA, cB = slice(hA * DH, (hA + 1) * DH), slice(hB * DH, (hB + 1) * DH)
        rnW = pn.tile([P, 16], F32, tag="rnW", name=f"rnW{p}")
        rnU = pn.tile([P, 16], F32, tag="rnU", name=f"rnU{p}")
        ET = [[None] * ICH, [None] * ICH]
        E = [[None] * ICH, [None] * ICH]

        ups = psUW.tile([P, N], F32, tag="uw", name=f"ups{p}")

        def u_step(jc):
            lA, lB = CV[jc][:, cA], CV[jc][:, cB]
            st, sp = (jc == 0), (jc == ICH - 1)
            nc.tensor.matmul(ups[0:64, 0:512], lA, ET[0][jc][:, 0:512], start=st, stop=sp)
            nc.tensor.matmul(ups[64:128, 0:512], lB, ET[1][jc][:, 0:512], start=st, stop=sp)
            nc.tensor.matmul(ups[0:64, 512:1024], lA, ET[0][jc][:, 512:1024], start=st, stop=sp)
            nc.tensor.matmul(ups[64:128, 512:1024], lB, ET[1][jc][:, 512:1024], start=st, stop=sp)

        # --- ET loop: simT (rows = j-chunk, free = i), U laddered lag 2 ---
        for jc in range(ICH):
            if weave:
                weave.pop(0)()
            ps0 = psS.tile([P, N], F32, tag="sim", name=f"sT{p}_{jc}_0")
            ps1 = psS.tile([P, N], F32, tag="sim", name=f"sT{p}_{jc}_1")
            l0 = KTm[p][0:64, jc * P:(jc + 1) * P]
            l1 = KTm[p][64:128, jc * P:(jc + 1) * P]
            nc.tensor.matmul(ps0[:, 0:512], l0, QTm[p][0:64, 0:512])
            nc.tensor.matmul(ps1[:, 0:512], l1, QTm[p][64:128, 0:512])
            nc.tensor.matmul(ps0[:, 512:1024], l0, QTm[p][0:64, 512:1024])
            nc.tensor.matmul(ps1[:, 512:1024], l1, QTm[p][64:128, 512:1024])
            e0 = pe.tile([P, N], BF16, tag="ET")
            nc.scalar.activation(e0[:], ps0[:], EXP, scale=SCALE,
                                 accum_out=rnW[:, jc:jc + 1])
            ET[0][jc] = e0
            e1 = pe.tile([P, N], BF16, tag="ET")
            nc.scalar.activation(e1[:], ps1[:], EXP, scale=SCALE,
                                 accum_out=rnW[:, 8 + jc:9 + jc])
            ET[1][jc] = e1
            if jc >= 2:
                u_step(jc - 2)
        u_step(ICH - 2)
        u_step(ICH - 1)

        # free ups early: raw copy to SBUF; prep W normalizer broadcast
        u_raw = pub.tile([P, N], F32, tag="uraw", name=f"uraw{p}")
        nc.vector.tensor_copy(u_raw[:], ups[:])
        rrW = pn.tile([P, 16], F32, tag="rrW", name=f"rrW{p}")
        nc.vector.reciprocal_approx_fast(rrW[:], rnW[:])
        sw = 4 * p
        nc.sync.dma_start(
            T["normd"][sw:sw + 1, :].rearrange("o (c p) -> (o p) c", p=P),
            rrW[:, 0:8])
        nc.scalar.dma_start(
            T["normd"][sw + 1:sw + 2, :].rearrange("o (c p) -> (o p) c", p=P),
            rrW[:, 8:16])
        rbcW = pn.tile([P, N], F32, tag="rbcW", name=f"rbcW{p}")
        nc.sync.dma_start(rbcW[0:64, :],
                          T["normd"][sw:sw + 1, :].to_broadcast((64, N)))
        nc.scalar.dma_start(rbcW[64:128, :],
                            T["normd"][sw + 1:sw + 2, :].to_broadcast((64, N)))

        wps = psUW.tile([P, N], F32, tag="uw", name=f"wps{p}")

        def w_step(ic):
            lA, lB = V[ic][:, cA], V[ic][:, cB]
            st, sp = (ic == 0), (ic == ICH - 1)
            nc.tensor.matmul(wps[0:64, 0:512], lA, E[0][ic][:, 0:512], start=st, stop=sp)
            nc.tensor.matmul(wps[64:128, 0:512], lB, E[1][ic][:, 0:512], start=st, stop=sp)
            nc.tensor.matmul(wps[0:64, 512:1024], lA, E[0][ic][:, 512:1024], start=st, stop=sp)
            nc.tensor.matmul(wps[64:128, 512:1024], lB, E[1][ic][:, 512:1024], start=st, stop=sp)

        # --- E loop: sim (rows = i-chunk, free = j), W laddered lag 2 ---
        for ic in range(ICH):
            if weave:
                weave.pop(0)()
            ps0 = psS.tile([P, N], F32, tag="sim", name=f"sE{p}_{ic}_0")
            ps1 = psS.tile([P, N], F32, tag="sim", name=f"sE{p}_{ic}_1")
            l0 = QTm[p][0:64, ic * P:(ic + 1) * P]
            l1 = QTm[p][64:128, ic * P:(ic + 1) * P]
            nc.tensor.matmul(ps0[:, 0:512], l0, KTm[p][0:64, 0:512])
            nc.tensor.matmul(ps1[:, 0:512], l1, KTm[p][64:128, 0:512])
            nc.tensor.matmul(ps0[:, 512:1024], l0, KTm[p][0:64, 512:1024])
            nc.tensor.matmul(ps1[:, 512:1024], l1, KTm[p][64:128, 512:1024])
            e0 = pe.tile([P, N], BF16, tag="E")
            nc.scalar.activation(e0[:], ps0[:], EXP, scale=SCALE,
                                 accum_out=rnU[:, ic:ic + 1])
            E[0][ic] = e0
            e1 = pe.tile([P, N], BF16, tag="E")
            nc.scalar.activation(e1[:], ps1[:], EXP, scale=SCALE,
                                 accum_out=rnU[:, 8 + ic:9 + ic])
            E[1][ic] = e1
            if ic >= 2:
                w_step(ic - 2)
        w_step(ICH - 2)
        w_step(ICH - 1)

        # U normalizer broadcast (latency-first), then both muls
        rrU = pn.tile([P, 16], F32, tag="rrU", name=f"rrU{p}")
        nc.vector.reciprocal_approx_fast(rrU[:], rnU[:])
        su = 4 * p + 2
        nc.sync.dma_start(
            T["normd"][su:su + 1, :].rearrange("o (c p) -> (o p) c", p=P),
            rrU[:, 0:8])
        nc.scalar.dma_start(
            T["normd"][su + 1:su + 2, :].rearrange("o (c p) -> (o p) c", p=P),
            rrU[:, 8:16])
        rbcU = pn.tile([P, N], F32, tag="rbcU", name=f"rbcU{p}")
        nc.sync.dma_start(rbcU[0:64, :],
                          T["normd"][su:su + 1, :].to_broadcast((64, N)))
        nc.scalar.dma_start(rbcU[64:128, :],
                            T["normd"][su + 1:su + 2, :].to_broadcast((64, N)))

        wbf = pub.tile([P, N], BF16, tag="wbf", name=f"wbf{p}")
        nc.vector.tensor_mul(wbf[:], wps[:], rbcW[:])
        nc.scalar.dma_start(T["uwl"][p][128:256, :], wbf[:])
        ubf = pub.tile([P, N], BF16, tag="ubf", name=f"ubf{p}")
        nc.vector.tensor_mul(ubf[:], u_raw[:], rbcU[:])
        nc.sync.dma_start(T["uwl"][p][0:128, :], ubf[:])

        nc.gpsimd.collective_compute(
            "AllGather", mybir.AluOpType.bypass,
            replica_groups=GROUPS,
            ins=[T["uwl"][p][:]],
            outs=[T["uwa"][p][:]],
        )
        load_uw(2 * p, T["uwa"][p], 0, 128)
        load_uw(2 * p + 1, T["uwa"][p], 256, 384)

    # ---- prologue projections ----
    proj_qt("q", 0)
    proj_qt("k", 0)
    proj_v("c", 0)
    proj_v("c", 1)

    # ---- weave schedules ----
    w0 = [lambda ic=ic: proj_v("c", ic) for ic in range(2, 8)]
    w0 += [lambda ic=ic: proj_v("v", ic) for ic in range(0, 8)]
    w0 += [lambda: proj_qt("q", 1), lambda: proj_qt("k", 1)]
    w1 = [lambda: proj_qt("q", 2), lambda: proj_qt("k", 2)]
    w2 = [lambda: proj_qt("q", 3), lambda: proj_qt("k", 3)]

    pair_attention(0, w0)
    pair_attention(1, w1)
    pair_attention(2, w2)
    pair_attention(3, [])
    stmid.close()

    # ---- final projections (out needs all U; ctx early chunks interleaved) ----
    with tc.tile_pool(name="po", bufs=4) as po, \
         tc.tile_pool(name="psD", bufs=8, space="PSUM") as psD:
        ctx_part = []
        for ic in range(ICH):
            ps = psD.tile([P, COLS], F32, tag="od")
            for k in range(KCH):
                nc.tensor.matmul(ps[:], u_sb[k][:, ic * P:(ic + 1) * P],
                                 wout_sb[k][:],
                                 start=(k == 0), stop=(k == KCH - 1))
            o = po.tile([P, COLS], F32, tag="ot")
            nc.vector.tensor_add(o[:], ps[:], bout_bc[:])
            nc.sync.dma_start(T["out_cols"][ic * P:(ic + 1) * P, :], o[:])
            ps2 = psD.tile([P, COLS], F32, tag="od", name=f"ctxp{ic}")
            for k in range(6):
                nc.tensor.matmul(ps2[:], w_sb[k][:, ic * P:(ic + 1) * P],
                                 cwout_sb[k][:],
                                 start=(k == 0), stop=(k == 5))
            cp_t = pu.tile([P, COLS], F32, tag=f"cp{ic}")
            nc.vector.tensor_add(cp_t[:], ps2[:], cbout_bc[:])
            ctx_part.append(cp_t)
        for ic in range(ICH):
            ps = psD.tile([P, COLS], F32, tag="od", name=f"ctxf{ic}")
            for k in (6, 7):
                nc.tensor.matmul(ps[:], w_sb[k][:, ic * P:(ic + 1) * P],
                                 cwout_sb[k][:],
                                 start=(k == 6), stop=(k == 7))
            o = po.tile([P, COLS], F32, tag="ot")
            nc.vector.tensor_add(o[:], ps[:], ctx_part[ic][:])
            nc.scalar.dma_start(T["ctx_cols"][ic * P:(ic + 1) * P, :], o[:])
    stack.close()


def _get_nc():
    global _CACHED_NC
    if _CACHED_NC is None:
        _CACHED_NC = _build_nc()
    return _CACHED_NC


def _reorder_rows(w):
    """Reorder [INNER, :] rows to the uw_all K-chunk order (p-major, group X)."""
    chunks = []
    for p in range(4):
        for X in range(2):
            chunks.append(w[X * 512 + p * 128:X * 512 + (p + 1) * 128])
    return np.concatenate(chunks, axis=0)


def _m_major(w):
    """[1024, 512] -> [4096, 128]: m-block rows stacked."""
    return np.ascontiguousarray(
        w.reshape(DIM, 4, P).transpose(1, 0, 2).reshape(4 * DIM, P))


def kernel(x, context, w_qk, w_v, cw_qk, cw_v, w_out, b_out, cw_out, cb_out):
    x = np.asarray(x, dtype=np.float32)
    context = np.asarray(context, dtype=np.float32)
    w_qk = np.asarray(w_qk, dtype=np.float32)
    w_v = np.asarray(w_v, dtype=np.float32)
    cw_qk = np.asarray(cw_qk, dtype=np.float32)
    cw_v = np.asarray(cw_v, dtype=np.float32)
    w_out_r = _reorder_rows(np.asarray(w_out, dtype=np.float32)).astype(ml_dtypes.bfloat16)
    cw_out_r = _reorder_rows(np.asarray(cw_out, dtype=np.float32)).astype(ml_dtypes.bfloat16)
    b_out = np.asarray(b_out, dtype=np.float32)
    cb_out = np.asarray(cb_out, dtype=np.float32)

    in_maps = []
    for c in range(8):
        b, g = c // 2, c % 2
        sl = slice(g * IL, (g + 1) * IL)
        in_maps.append({
            "xT": np.ascontiguousarray(x[b].T).astype(ml_dtypes.bfloat16),
            "ctxT": np.ascontiguousarray(context[b].T).astype(ml_dtypes.bfloat16),
            "wqk": _m_major(w_qk[:, sl]).astype(ml_dtypes.bfloat16),
            "cwqk": _m_major(cw_qk[:, sl]).astype(ml_dtypes.bfloat16),
            "wv": np.ascontiguousarray(w_v[:, sl]).astype(ml_dtypes.bfloat16),
            "cwv": np.ascontiguousarray(cw_v[:, sl]).astype(ml_dtypes.bfloat16),
            "wout": np.ascontiguousarray(w_out_r[:, sl]),
            "cwout": np.ascontiguousarray(cw_out_r[:, sl]),
            "bout": np.ascontiguousarray(b_out[None, sl]),
            "cbout": np.ascontiguousarray(cb_out[None, sl]),
        })

    nc = _get_nc()
    res = run_bass_kernel_spmd(nc, in_maps, list(range(8)))

    out = np.empty((B, N, DIM), dtype=np.float32)
    ctx_out = np.empty((B, N, DIM), dtype=np.float32)
    for b in range(B):
        out[b, :, 0:COLS] = res.results[2 * b]["out_cols"]
        out[b, :, COLS:] = res.results[2 * b + 1]["out_cols"]
        ctx_out[b, :, 0:COLS] = res.results[2 * b]["ctx_cols"]
        ctx_out[b, :, COLS:] = res.results[2 * b + 1]["ctx_cols"]
    return out, ctx_out


# revision 10
# speedup vs baseline: 1.5558x; 1.5558x over previous
"""Bidirectional cross-attention kernel for 8 Trainium2 NeuronCores.

Sharding: core c = 2*b + g handles batch b with head-group g (8 of 16 heads).
Compute structure matches the proven baseline (zero-padded K=128 sim matmuls,
M=65 ones-column U/W ladders, DMA-broadcast softmax normalization), but the
EMISSION ORDER is restructured for overlap:
 - Q/K projections run m-outer (head-pair-major), so pair 0's attention is
   emitted right after QT/KT m=0 instead of after all projections.  The exp
   pipeline starts ~60us earlier.
 - CV/V projections are WOVEN between pair-0's attention chunks so the PE
   back-fills the exp-shadow instead of idling.
 - Projection psums share the sim psum pool (PSUM stays within 8 banks).
 - input pools close after pair 0 to make room for late-phase tiles.
"""

import os
import sys

import numpy as np

for _p in ("/opt/trn_rl_repo", "/root/.axon_site/_ro/trn_rl_repo"):
    if os.path.isdir(_p) and _p not in sys.path:
        sys.path.append(_p)

import ml_dtypes  # noqa: E402
import concourse.bass as bass  # noqa: E402
import concourse.mybir as mybir  # noqa: E402
import concourse.tile as tile  # noqa: E402
from concourse import bacc  # noqa: E402
from concourse.bass_utils import run_bass_kernel_spmd  # noqa: E402

B, N, DIM = 4, 1024, 1024
H, DH = 16, 64
HL = 8            # heads per core
IL = HL * DH      # local inner width (512)
COLS = 512        # output columns per core
P = 128
PAIRS = HL // 2   # head pairs per core
KCH = DIM // P    # contraction chunks (8)
ICH = N // P      # sequence chunks (8)
SCALE = DH ** -0.5
GROUPS = [[0, 1], [2, 3], [4, 5], [6, 7]]

F32 = mybir.dt.float32
BF16 = mybir.dt.bfloat16
EXP = mybir.ActivationFunctionType.Exp

_CACHED_NC = None


def _build_nc():
    nc = bacc.Bacc("TRN2", target_bir_lowering=False, debug=False, num_devices=8)

    xT = nc.dram_tensor("xT", [DIM, N], BF16, kind="ExternalInput")
    ctxT = nc.dram_tensor("ctxT", [DIM, N], BF16, kind="ExternalInput")
    wqk = nc.dram_tensor("wqk", [DIM, IL], BF16, kind="ExternalInput")
    wv = nc.dram_tensor("wv", [DIM, IL], BF16, kind="ExternalInput")
    cwqk = nc.dram_tensor("cwqk", [DIM, IL], BF16, kind="ExternalInput")
    cwv = nc.dram_tensor("cwv", [DIM, IL], BF16, kind="ExternalInput")
    wout = nc.dram_tensor("wout", [DIM, COLS], BF16, kind="ExternalInput")
    cwout = nc.dram_tensor("cwout", [DIM, COLS], BF16, kind="ExternalInput")
    bout = nc.dram_tensor("bout", [1, COLS], F32, kind="ExternalInput")
    cbout = nc.dram_tensor("cbout", [1, COLS], F32, kind="ExternalInput")

    out_cols = nc.dram_tensor("out_cols", [N, COLS], F32, kind="ExternalOutput")
    ctx_cols = nc.dram_tensor("ctx_cols", [N, COLS], F32, kind="ExternalOutput")

    with tile.TileContext(nc) as tc:
        with tc.tile_pool(name="dram", bufs=1, space="DRAM") as dpool:
            uwl = [dpool.tile([256, N], BF16, tag=f"uwl{p}", name=f"uwl{p}")
                   for p in range(4)]
            uwa = [dpool.tile([512, N], BF16, tag=f"uwa{p}", name=f"uwa{p}")
                   for p in range(3)]
            uwa3u = dpool.tile([256, N], BF16, tag="uwa3u")
            uwa3w = dpool.tile([256, N], BF16, tag="uwa3w")
            normd = dpool.tile([16, N], F32, tag="normd")
            _build_body(nc, tc, dict(
                xT=xT, ctxT=ctxT, wqk=wqk, wv=wv, cwqk=cwqk, cwv=cwv,
                wout=wout, cwout=cwout, bout=bout, cbout=cbout,
                out_cols=out_cols, ctx_cols=ctx_cols,
                uwl=uwl, uwa=uwa, uwa3u=uwa3u, uwa3w=uwa3w, normd=normd,
            ))
    nc.compile()
    if os.environ.get("KERNEL_LDW_DEDUP", "1") == "1":
        _dedupe_ldweights(nc)
    return nc


def _dedupe_ldweights(nc):
    """Drop PE Ldweights that reload the exact weights already resident."""
    def sig(i):
        a = i.ins[0]
        return (a.memref, a.offset, str(a.ap), str(a.dtype),
                str(i.tile_position), str(i.tile_size),
                str(i.perf_mode), str(i.is_transpose))

    removed = 0
    for fn in nc.m.functions:
        for bb in fn.blocks:
            last = None
            keep = []
            for i in bb.instructions:
                if isinstance(i, mybir.InstLdweights):
                    s = sig(i)
                    si = i.sync_info
                    if s == last and (si is None or
                                      (not si.on_wait and not si.on_update)):
                        removed += 1
                        continue
                    last = s
                elif isinstance(i, mybir.InstMatmult):
                    pass
                elif getattr(i, "engine", None) == mybir.EngineType.PE:
                    last = None
                keep.append(i)
            if removed:
                bb.instructions = keep
    return removed


def _build_body(nc, tc, T):
    hs = 65  # head stride in the V/CV tiles (64 values + ones column)
    from contextlib import ExitStack
    stack = ExitStack()
    pqk = stack.enter_context(tc.tile_pool(name="pqk", bufs=1))
    pv = stack.enter_context(tc.tile_pool(name="pv", bufs=1))
    pf = stack.enter_context(tc.tile_pool(name="pf", bufs=1))
    pu = stack.enter_context(tc.tile_pool(name="pu", bufs=1))
    pe = stack.enter_context(tc.tile_pool(name="pe", bufs=5))
    pn = stack.enter_context(tc.tile_pool(name="pn", bufs=2))
    # closed after pair 0 (inputs consumed by then)
    stin = ExitStack()
    pin = stin.enter_context(tc.tile_pool(name="pin", bufs=1))
    pw = stin.enter_context(tc.tile_pool(name="pw", bufs=1))
    # psum pools: sim/proj share one tag pool (4 banks), uw ladders (4 banks)
    stmid = ExitStack()
    psS = stmid.enter_context(tc.tile_pool(name="psS", bufs=2, space="PSUM"))
    psUW = stmid.enter_context(tc.tile_pool(name="psUW", bufs=2, space="PSUM"))

    # ---- ACT table warmup so the exp table set loads during input DMA ----
    warm = pn.tile([1, 8], F32, tag="warm")
    nc.vector.memset(warm[:], 0.0)
    warm2 = pn.tile([1, 8], F32, tag="warm2")
    nc.scalar.activation(warm2[:], warm[:], EXP)

    # ---- input DMAs: x-side on sync queue, context-side on scalar queue ----
    xt, ct, wqk_t, cwqk_t, wv_t, cwv_t = [], [], [], [], [], []
    for k in range(KCH):
        w = pw.tile([P, IL], BF16, tag=f"wq{k}")
        nc.sync.dma_start(w[:], T["wqk"][k * P:(k + 1) * P, :])
        wqk_t.append(w)
        t = pin.tile([P, N], BF16, tag=f"xT{k}")
        nc.sync.dma_start(t[:], T["xT"][k * P:(k + 1) * P, :])
        xt.append(t)
        w = pw.tile([P, IL], BF16, tag=f"cwq{k}")
        nc.scalar.dma_start(w[:], T["cwqk"][k * P:(k + 1) * P, :])
        cwqk_t.append(w)
        t = pin.tile([P, N], BF16, tag=f"cT{k}")
        nc.scalar.dma_start(t[:], T["ctxT"][k * P:(k + 1) * P, :])
        ct.append(t)
    for k in range(KCH):
        w = pw.tile([P, IL], BF16, tag=f"wv{k}")
        nc.sync.dma_start(w[:], T["wv"][k * P:(k + 1) * P, :])
        wv_t.append(w)
        w = pw.tile([P, IL], BF16, tag=f"cwv{k}")
        nc.scalar.dma_start(w[:], T["cwv"][k * P:(k + 1) * P, :])
        cwv_t.append(w)
    bout_bc = pf.tile([P, COLS], F32, tag="bb")
    nc.sync.dma_start(bout_bc[:], T["bout"][:].to_broadcast((P, COLS)))
    cbout_bc = pf.tile([P, COLS], F32, tag="cbb")
    nc.scalar.dma_start(cbout_bc[:], T["cbout"][:].to_broadcast((P, COLS)))

    QT = [None] * 4   # per pair: (pa, pb) zero-padded halves
    KT = [None] * 4
    V = [None] * ICH
    CV = [None] * ICH

    def proj_qt(which, m):
        """QT/KT pair m: [128, N] psum -> zero-padded pa/pb tiles."""
        src, wtiles = (xt, wqk_t) if which == "q" else (ct, cwqk_t)
        ps = psS.tile([P, N], F32, tag="sim", name=f"pj{which}{m}")
        for k in range(KCH):
            lhsT = wtiles[k][:, m * P:(m + 1) * P]
            nc.tensor.matmul(ps[:, 0:512], lhsT, src[k][:, 0:512],
                             start=(k == 0), stop=(k == KCH - 1))
            nc.tensor.matmul(ps[:, 512:1024], lhsT, src[k][:, 512:1024],
                             start=(k == 0), stop=(k == KCH - 1))
        pa = pqk.tile([P, N], BF16, tag=f"{which}a{m}")
        nc.vector.tensor_copy(pa[0:DH, :], ps[0:DH, :])
        nc.vector.memset(pa[DH:P, :], 0.0)
        pb = pqk.tile([P, N], BF16, tag=f"{which}b{m}")
        nc.vector.memset(pb[0:DH, :], 0.0)
        nc.vector.tensor_copy(pb[DH:P, :], ps[DH:P, :])
        if which == "q":
            QT[m] = (pa, pb)
        else:
            KT[m] = (pa, pb)

    def proj_v(which, ic):
        """V/CV seq-chunk ic: [128, 8*65] bf16, head-strided + ones column."""
        src, wtiles = (xt, wv_t) if which == "v" else (ct, cwv_t)
        psf = psS.tile([P, N], F32, tag="sim", name=f"pj{which}{ic}")
        ps = psf[:, 0:IL]
        for k in range(KCH):
            nc.tensor.matmul(ps[:], src[k][:, ic * P:(ic + 1) * P], wtiles[k][:],
                             start=(k == 0), stop=(k == KCH - 1))
        o = pv.tile([P, HL * hs], BF16, tag=f"{which}{ic}")
        dst = o[:].rearrange("p (h e) -> p h e", e=hs)
        nc.vector.tensor_copy(dst[:, :, 0:DH],
                              ps[:].rearrange("p (h e) -> p h e", e=DH))
        nc.vector.memset(dst[:, :, DH:hs], 1.0)
        if which == "v":
            V[ic] = o
        else:
            CV[ic] = o

    u_sb = [None] * KCH
    w_sb = [None] * KCH

    def load_uw(k, src_tile, u_off, w_off):
        usrc = src_tile if src_tile is not None else T["uwa3u"]
        wsrc = src_tile if src_tile is not None else T["uwa3w"]
        t = pu.tile([P, N], BF16, tag=f"ua{k}")
        nc.sync.dma_start(t[:], usrc[u_off:u_off + P, :])
        u_sb[k] = t
        t = pu.tile([P, N], BF16, tag=f"wa{k}")
        nc.scalar.dma_start(t[:], wsrc[w_off:w_off + P, :])
        w_sb[k] = t

    def pair_attention(p, weave):
        E = [[None] * ICH, [None] * ICH]
        ET = [[None] * ICH, [None] * ICH]

        def norm_store(psum, slot, dst, dst_row):
            rst = pn.tile([DH + 1, N], F32, tag="rst")
            nc.vector.tensor_copy(rst[:], psum[0:DH + 1, :])
            nc.sync.dma_start(T["normd"][slot:slot + 1, :], rst[DH:DH + 1, :])
            rbc = pn.tile([DH, N], F32, tag="rbc")
            nc.sync.dma_start(
                rbc[:], T["normd"][slot:slot + 1, :].to_broadcast((DH, N)))
            nc.vector.reciprocal_approx_fast(rbc[:], rbc[:])
            ubf = pn.tile([DH, N], BF16, tag="ubf")
            nc.vector.tensor_mul(ubf[:], rst[0:DH, :], rbc[:])
            nc.sync.dma_start(dst[dst_row:dst_row + DH, :], ubf[:])

        # --- simT -> ET, with U-accumulation laddered in (lag 2) ---
        ups = [psUW.tile([P, N], F32, tag="uw", name=f"ups{p}_{hh}")
               for hh in range(2)]

        def u_step(hh, jc):
            h = 2 * p + hh
            lhsT = CV[jc][:, h * hs:(h + 1) * hs]
            nc.tensor.matmul(ups[hh][0:hs, 0:512], lhsT, ET[hh][jc][:, 0:512],
                             start=(jc == 0), stop=(jc == ICH - 1))
            nc.tensor.matmul(ups[hh][0:hs, 512:1024], lhsT,
                             ET[hh][jc][:, 512:1024],
                             start=(jc == 0), stop=(jc == ICH - 1))

        for jc in range(ICH):
            if weave:
                weave.pop(0)()
            for hh in range(2):
                ps = psS.tile([P, N], F32, tag="sim")
                lhsT = KT[p][hh][:, jc * P:(jc + 1) * P]
                nc.tensor.matmul(ps[:, 0:512], lhsT, QT[p][hh][:, 0:512],
                                 start=True, stop=True)
                nc.tensor.matmul(ps[:, 512:1024], lhsT, QT[p][hh][:, 512:1024],
                                 start=True, stop=True)
                e = pe.tile([P, N], BF16, tag="ET")
                nc.scalar.activation(e[:], ps[:], EXP, scale=SCALE)
                ET[hh][jc] = e
            if jc >= 2:
                for hh in range(2):
                    u_step(hh, jc - 2)
        for jc in (ICH - 2, ICH - 1):
            for hh in range(2):
                u_step(hh, jc)
        for hh in range(2):
            norm_store(ups[hh], p * 4 + hh, T["uwl"][p], hh * DH)

        if p == 3:
            nc.gpsimd.collective_compute(
                "AllGather", mybir.AluOpType.bypass,
                replica_groups=GROUPS,
                ins=[T["uwl"][3][0:128, :]],
                outs=[T["uwa3u"][:]],
            )

        # --- sim -> E, with W-accumulation laddered in (lag 2) ---
        wps = [psUW.tile([P, N], F32, tag="uw", name=f"wps{p}_{hh}")
               for hh in range(2)]

        def w_step(hh, ic):
            h = 2 * p + hh
            lhsT = V[ic][:, h * hs:(h + 1) * hs]
            nc.tensor.matmul(wps[hh][0:hs, 0:512], lhsT, E[hh][ic][:, 0:512],
                             start=(ic == 0), stop=(ic == ICH - 1))
            nc.tensor.matmul(wps[hh][0:hs, 512:1024], lhsT,
                             E[hh][ic][:, 512:1024],
                             start=(ic == 0), stop=(ic == ICH - 1))

        for ic in range(ICH):
            if weave:
                weave.pop(0)()
            for hh in range(2):
                ps = psS.tile([P, N], F32, tag="sim")
                lhsT = QT[p][hh][:, ic * P:(ic + 1) * P]
                nc.tensor.matmul(ps[:, 0:512], lhsT, KT[p][hh][:, 0:512],
                                 start=True, stop=True)
                nc.tensor.matmul(ps[:, 512:1024], lhsT, KT[p][hh][:, 512:1024],
                                 start=True, stop=True)
                e = pe.tile([P, N], BF16, tag="E")
                nc.scalar.activation(e[:], ps[:], EXP, scale=SCALE)
                E[hh][ic] = e
            if ic >= 2:
                for hh in range(2):
                    w_step(hh, ic - 2)
        for ic in (ICH - 2, ICH - 1):
            for hh in range(2):
                w_step(hh, ic)
        for hh in range(2):
            norm_store(wps[hh], p * 4 + 2 + hh, T["uwl"][p], 128 + hh * DH)

        if p < 3:
            nc.gpsimd.collective_compute(
                "AllGather", mybir.AluOpType.bypass,
                replica_groups=GROUPS,
                ins=[T["uwl"][p][:]],
                outs=[T["uwa"][p][:]],
            )
            load_uw(2 * p, T["uwa"][p], 0, 128)
            load_uw(2 * p + 1, T["uwa"][p], 256, 384)
        else:
            nc.gpsimd.collective_compute(
                "AllGather", mybir.AluOpType.bypass,
                replica_groups=GROUPS,
                ins=[T["uwl"][3][128:256, :]],
                outs=[T["uwa3w"][:]],
            )
            load_uw(6, None, 0, 0)
            load_uw(7, None, 128, 128)

    # ---- prologue: all QT/KT pairs + first CV chunks ----
    proj_qt("q", 0)
    proj_qt("k", 0)
    proj_qt("q", 1)
    proj_qt("k", 1)
    proj_qt("q", 2)
    proj_qt("k", 2)
    proj_qt("q", 3)
    proj_qt("k", 3)
    proj_v("c", 0)
    proj_v("c", 1)

    # pair 0 absorbs the remaining CV/V projections in its exp-shadow
    if os.environ.get("KERNEL_WEAVE", "1") == "1":
        w0 = [lambda ic=ic: proj_v("c", ic) for ic in range(2, 8)]
        w0 += [lambda ic=ic: proj_v("v", ic) for ic in range(0, 8)]
    else:
        for ic in range(2, 8):
            proj_v("c", ic)
        for ic in range(0, 8):
            proj_v("v", ic)
        w0 = []

    pair_attention(0, w0)
    stin.close()
    # output-projection weights: loaded during pair 1
    wout_sb, cwout_sb = [], []
    for k in range(KCH):
        t = pf.tile([P, COLS], BF16, tag=f"wo{k}")
        nc.sync.dma_start(t[:], T["wout"][k * P:(k + 1) * P, :])
        wout_sb.append(t)
        t = pf.tile([P, COLS], BF16, tag=f"cwo{k}")
        nc.scalar.dma_start(t[:], T["cwout"][k * P:(k + 1) * P, :])
        cwout_sb.append(t)
    pair_attention(1, [])
    pair_attention(2, [])
    pair_attention(3, [])
    stmid.close()

    # ---- final projections (out needs all U; ctx split 6+2) ----
    with tc.tile_pool(name="po", bufs=4) as po, \
         tc.tile_pool(name="pcp", bufs=1) as pcp, \
         tc.tile_pool(name="psD", bufs=8, space="PSUM") as psD:
        ctx_part = []
        for ic in range(ICH):
            ps = psD.tile([P, COLS], F32, tag="od")
            for k in range(KCH):
                nc.tensor.matmul(ps[:], u_sb[k][:, ic * P:(ic + 1) * P],
                                 wout_sb[k][:],
                                 start=(k == 0), stop=(k == KCH - 1))
            o = po.tile([P, COLS], F32, tag="ot")
            nc.vector.tensor_add(o[:], ps[:], bout_bc[:])
            nc.sync.dma_start(T["out_cols"][ic * P:(ic + 1) * P, :], o[:])
            ps2 = psD.tile([P, COLS], F32, tag="od", name=f"ctxp{ic}")
            for k in range(6):
                nc.tensor.matmul(ps2[:], w_sb[k][:, ic * P:(ic + 1) * P],
                                 cwout_sb[k][:],
                                 start=(k == 0), stop=(k == 5))
            cp_t = pcp.tile([P, COLS], F32, tag=f"cp{ic}")
            nc.vector.tensor_add(cp_t[:], ps2[:], cbout_bc[:])
            ctx_part.append(cp_t)
        for ic in range(ICH):
            ps = psD.tile([P, COLS], F32, tag="od", name=f"ctxf{ic}")
            for k in (6, 7):
                nc.tensor.matmul(ps[:], w_sb[k][:, ic * P:(ic + 1) * P],
                                 cwout_sb[k][:],
                                 start=(k == 6), stop=(k == 7))
            o = po.tile([P, COLS], F32, tag="ot")
            nc.vector.tensor_add(o[:], ps[:], ctx_part[ic][:])
            nc.scalar.dma_start(T["ctx_cols"][ic * P:(ic + 1) * P, :], o[:])
    stack.close()


def _get_nc():
    global _CACHED_NC
    if _CACHED_NC is None:
        _CACHED_NC = _build_nc()
    return _CACHED_NC


def _reorder_rows(w):
    """Reorder [INNER, :] rows to the uw_all K-chunk order (p-major, group X)."""
    chunks = []
    for p in range(4):
        for X in range(2):
            chunks.append(w[X * 512 + p * 128:X * 512 + (p + 1) * 128])
    return np.concatenate(chunks, axis=0)


def kernel(x, context, w_qk, w_v, cw_qk, cw_v, w_out, b_out, cw_out, cb_out):
    x = np.asarray(x, dtype=np.float32)
    context = np.asarray(context, dtype=np.float32)
    w_qk = np.asarray(w_qk, dtype=np.float32)
    w_v = np.asarray(w_v, dtype=np.float32)
    cw_qk = np.asarray(cw_qk, dtype=np.float32)
    cw_v = np.asarray(cw_v, dtype=np.float32)
    w_out_r = _reorder_rows(np.asarray(w_out, dtype=np.float32)).astype(ml_dtypes.bfloat16)
    cw_out_r = _reorder_rows(np.asarray(cw_out, dtype=np.float32)).astype(ml_dtypes.bfloat16)
    b_out = np.asarray(b_out, dtype=np.float32)
    cb_out = np.asarray(cb_out, dtype=np.float32)

    in_maps = []
    for c in range(8):
        b, g = c // 2, c % 2
        sl = slice(g * IL, (g + 1) * IL)
        in_maps.append({
            "xT": np.ascontiguousarray(x[b].T).astype(ml_dtypes.bfloat16),
            "ctxT": np.ascontiguousarray(context[b].T).astype(ml_dtypes.bfloat16),
            "wqk": np.ascontiguousarray(w_qk[:, sl]).astype(ml_dtypes.bfloat16),
            "wv": np.ascontiguousarray(w_v[:, sl]).astype(ml_dtypes.bfloat16),
            "cwqk": np.ascontiguousarray(cw_qk[:, sl]).astype(ml_dtypes.bfloat16),
            "cwv": np.ascontiguousarray(cw_v[:, sl]).astype(ml_dtypes.bfloat16),
            "wout": np.ascontiguousarray(w_out_r[:, sl]),
            "cwout": np.ascontiguousarray(cw_out_r[:, sl]),
            "bout": np.ascontiguousarray(b_out[None, sl]),
            "cbout": np.ascontiguousarray(cb_out[None, sl]),
        })

    nc = _get_nc()
    res = run_bass_kernel_spmd(nc, in_maps, list(range(8)))

    out = np.empty((B, N, DIM), dtype=np.float32)
    ctx_out = np.empty((B, N, DIM), dtype=np.float32)
    for b in range(B):
        out[b, :, 0:COLS] = res.results[2 * b]["out_cols"]
        out[b, :, COLS:] = res.results[2 * b + 1]["out_cols"]
        ctx_out[b, :, 0:COLS] = res.results[2 * b]["ctx_cols"]
        ctx_out[b, :, COLS:] = res.results[2 * b + 1]["ctx_cols"]
    return out, ctx_out
